# revision 17
# baseline (speedup 1.0000x reference)
"""DoRA linear kernel for 8 Trainium2 NeuronCores.

out = (base_output + 2.0 * x @ lora_A^T @ lora_B^T) * magnitude / (||base_weight + 2.0 * lora_B @ lora_A||_row + eps)

Sharding (row-parallel hint):
  - tokens (B*S = 8192) data-parallel: 1024 per core (x, base_output, out)
  - base_weight / magnitude row-parallel: 512 out_features per core; the
    per-row norm is fully local, mag_scale is allgathered (16KB collective)
  - lora_A / lora_B replicated

Key design points (all layout transforms done on host):
  - x shipped TRANSPOSED (d-major): stage 1 (xa = 2A @ x^T) needs no PE
    transposes.
  - base_output / out transposed (out_features on partitions): the mag
    rescale is a per-partition DVE tensor_scalar, and the base add costs
    ZERO engine cycles -- a gpsimd software-DGE DMA with accum_op=add
    accumulates base^T straight into the delta tile in SBUF.
  - stage-0 square+rowsum runs on DVE via tensor_tensor_reduce (accum_out),
    keeping ACT free for the epilogue PSUM->SBUF copies.
  - base/out bf16, W fp8-e4m3 scaled by 64 (range fix): 49.5 -> 27.6MB HBM.
  - All tiny-descriptor DMAs eliminated (host pre-tiles magsh; the mag
    collective in/out goes through DVE 32x32 block transposes so every DMA
    runs >= 512B-contiguous descriptors).
  - The collective is triggered as soon as stage 0 drains (~25us) so the
    mag-gated tail (DVE scale + stores) rarely waits.

Engine FIFOs (order == emission order per engine):
  sync : magsh b2s a2 at2 x*8 | maglin | stores*32
  ACT  : W*2 b2f | sqrt | xa copies | comb copies*32
  DVE  : stage0 ttr*16, ss reduce, tail | magb transposes | scale*32
  gpsimd: ident8, cc_in, AllGather | base accum-DMA*32
  PE   : stage0 mm*32, stage1 mm*64, stage2 mm*64
"""

import sys

sys.path.insert(0, "/opt/trn_rl_repo")

import ml_dtypes
import numpy as np

import concourse.bass as bass  # noqa: F401
import concourse.mybir as mybir
import concourse.tile as tile
from concourse import bacc
from concourse.bass_utils import run_bass_kernel_spmd
from concourse.masks import make_identity

N_CORES = 8
T, D, O, R = 8192, 4096, 4096, 64
T_LOC = T // N_CORES  # 1024 tokens per core
O_SH = O // N_CORES  # 512 weight rows per core
SCALING = 2.0
EPS = 1e-8
W_SC = 64.0  # fp8 pre-scale for W (and matching 64x on stage-0 A, mag)
F32 = mybir.dt.float32
BF16 = mybir.dt.bfloat16
FP8 = mybir.dt.float8e4
NP_BF16 = ml_dtypes.bfloat16
NP_FP8 = ml_dtypes.float8_e4m3fn

ACCUM_BASE = False  # add base^T via gpsimd accum-DMA (True) or DVE add (False)
N_OC = O // 128  # 32 global o-chunks (epilogue)
N_OCL = O_SH // 128  # 4 local o-chunks (stage 0)
N_DC = D // 128  # 32 d-chunks (stage 1)
N_XC = 8  # x dma chunks (512 d-rows each)

_CACHE: dict = {}


def _emit(nc, tc, aps):
    xt_d = aps["xt"]  # [8, 128, 4096] bf16  x^T chunks
    bt_d = aps["bt"]  # [32, 128, 1024] bf16 base^T per-oc tiles
    wt_d = aps["wt"]  # [128, 16384] fp8     64*W rows as [128, 4 ocl, 4096]
    a2_d = aps["a2"]  # [64, 4096] bf16      128*A (stage-0 rhs)
    at2_d = aps["at2"]  # [128, 2048] bf16   (2A)^T chunks (stage-1 lhsT)
    b2f_d = aps["b2f"]  # [64, 4096] bf16    B^T full
    b2s_d = aps["b2s"]  # [64, 512] bf16     B^T local o-shard
    mags_d = aps["mags"]  # [128, 4] f32     64*magnitude shard (host-tiled)
    out_d = aps["outT"]  # [32, 128, 1024] bf16 out^T tiles

    import contextlib

    ctx = contextlib.ExitStack()
    with ctx:
        const = ctx.enter_context(tc.tile_pool(name="const", bufs=1))
        combpool = ctx.enter_context(tc.tile_pool(name="combpool", bufs=12))
        ps = ctx.enter_context(tc.tile_pool(name="ps", bufs=3, space="PSUM"))
        pxa = ctx.enter_context(tc.tile_pool(name="pxa", bufs=1, space="PSUM"))
        dram = ctx.enter_context(tc.tile_pool(name="dram", bufs=1, space="DRAM"))

        # ---- phase 0: input DMA triggers
        # sync ring: stage0/1 lora consts, then x^T chunks (8MB)
        b2s_sb = const.tile([64, O_SH], BF16)
        nc.sync.dma_start(b2s_sb[:], b2s_d[:])
        a2_sb = const.tile([64, D], BF16)
        nc.sync.dma_start(a2_sb[:], a2_d[:])
        at2_sb = const.tile([128, N_DC * R], BF16)
        nc.sync.dma_start(at2_sb[:], at2_d[:])
        magsh_sb = const.tile([128, 4], F32)
        nc.sync.dma_start(magsh_sb[:], mags_d[:])
        xt_sb = []
        for g in range(N_XC):
            t = const.tile([128, 4096], BF16, name=f"xt_{g}")
            nc.sync.dma_start(t[:], xt_d[g])
            xt_sb.append(t)

        # scalar ring: W first (gates the collective), then b2f
        w_sb = const.tile([128, N_OCL * D], FP8)
        nc.scalar.dma_start(w_sb[:, 0 : 2 * D], wt_d[:, 0 : 2 * D])
        nc.scalar.dma_start(w_sb[:, 2 * D : 4 * D], wt_d[:, 2 * D : 4 * D])
        b2f_sb = const.tile([64, O], BF16)
        nc.scalar.dma_start(b2f_sb[:], b2f_d[:])
        bt_sb = []
        if not ACCUM_BASE:
            bt_r = bt_d.rearrange("(c o) p t -> c p o t", o=4)
            for cb in range(N_XC):
                t = const.tile([128, 4, T_LOC], BF16, name=f"bt_{cb}")
                nc.scalar.dma_start(t[:], bt_r[cb])
                bt_sb.append(t)

        ident8 = const.tile([128, 128], FP8)
        make_identity(nc, ident8[:])

        # ---- stage 0: ss = ||64*(W + 2BA)||^2 per local row -> mag_scale
        # dc-pair psum tiles [128,1024]; ACT squares straight from PSUM with
        # the accumulator producing the row-sums (single-bank reads only)
        ss_sb = const.tile([128, 32], F32)
        sqpool = ctx.enter_context(tc.tile_pool(name="sqpool", bufs=3))
        for ocl in range(N_OCL):
            for t4 in range(4):
                pu = ps.tile([128, 1024], F32, tag="ps", name=f"pu_{ocl}_{t4}")
                for h in range(2):
                    dc = 2 * t4 + h
                    nc.tensor.matmul(
                        pu[:, 512 * h : 512 * (h + 1)],
                        b2s_sb[:, 128 * ocl : 128 * (ocl + 1)],
                        a2_sb[:, 512 * dc : 512 * (dc + 1)],
                        start=True,
                        stop=False,
                        skip_group_check=True,
                    )
                for h in range(2):
                    dc = 2 * t4 + h
                    nc.tensor.matmul(
                        pu[:, 512 * h : 512 * (h + 1)],
                        ident8[:],
                        w_sb[:, D * ocl + 512 * dc : D * ocl + 512 * (dc + 1)],
                        start=False,
                        stop=True,
                        skip_group_check=True,
                    )
                for h in range(2):
                    sq = sqpool.tile(
                        [128, 512], BF16, tag="sq", name=f"sq_{ocl}_{t4}_{h}"
                    )
                    k = 8 * ocl + 2 * t4 + h
                    nc.scalar.activation(
                        sq[:],
                        pu[:, 512 * h : 512 * (h + 1)],
                        mybir.ActivationFunctionType.Square,
                        accum_out=ss_sb[:, k : k + 1],
                    )

        # tail: magsc = (64*mag) / (sqrt(ss) + 64*eps), then allgather
        ssr_sb = const.tile([128, N_OCL], F32)
        for ocl in range(N_OCL):
            nc.vector.tensor_reduce(
                ssr_sb[:, ocl : ocl + 1],
                ss_sb[:, 8 * ocl : 8 * (ocl + 1)],
                axis=mybir.AxisListType.X,
                op=mybir.AluOpType.add,
            )
        nrm_sb = const.tile([128, N_OCL], F32)
        nc.scalar.sqrt(nrm_sb[:], ssr_sb[:])
        nc.vector.tensor_scalar_add(nrm_sb[:], nrm_sb[:], W_SC * EPS)
        rinv_sb = const.tile([128, N_OCL], F32)
        nc.vector.reciprocal(rinv_sb[:], nrm_sb[:])
        magsc_sb = const.tile([128, N_OCL], F32)
        nc.vector.tensor_tensor(
            out=magsc_sb[:],
            in0=rinv_sb[:],
            in1=magsh_sb[:],
            op=mybir.AluOpType.mult,
        )
        cc_in = dram.tile([O_SH], F32)
        cc_out = dram.tile([O], F32, addr_space="Shared")
        nc.gpsimd.dma_start(cc_in.rearrange("(oc p) -> p oc", p=128), magsc_sb[:])
        nc.gpsimd.collective_compute(
            "AllGather",
            mybir.AluOpType.bypass,
            replica_groups=[list(range(N_CORES))],
            ins=[cc_in[:]],
            outs=[cc_out[:]],
        )
        # [4096] -> [32,128] contiguous load, then block-transpose to [128,32]
        maglin_sb = const.tile([32, 128], F32)
        nc.sync.dma_start(maglin_sb[:], cc_out.rearrange("(q f) -> q f", f=128))
        magb_sb = const.tile([128, N_OC], F32)
        for b in range(4):
            nc.vector.transpose(
                magb_sb[32 * b : 32 * (b + 1), 0:32],
                maglin_sb[0:32, 32 * b : 32 * (b + 1)],
            )

        # ---- stage 1: xa^T[64, 1024] = (2A) @ x^T, accumulated over d
        pxa_t = pxa.tile([64, 1024], F32, name="pxa01")
        pxa0 = pxa_t[:, 0:512]
        pxa1 = pxa_t[:, 512:1024]
        for g in range(N_XC):
            for j in range(4):
                dc = 4 * g + j
                lhsT = at2_sb[:, R * dc : R * (dc + 1)]
                nc.tensor.matmul(
                    pxa0,
                    lhsT,
                    xt_sb[g][:, 1024 * j : 1024 * j + 512],
                    start=(dc == 0),
                    stop=(dc == N_DC - 1),
                )
                nc.tensor.matmul(
                    pxa1,
                    lhsT,
                    xt_sb[g][:, 1024 * j + 512 : 1024 * (j + 1)],
                    start=(dc == 0),
                    stop=(dc == N_DC - 1),
                )
        xaT_sb = const.tile([64, 1024], BF16)
        nc.scalar.copy(xaT_sb[:, 0:512], pxa0)
        nc.scalar.copy(xaT_sb[:, 512:1024], pxa1)

        # ---- stage 2 epilogue, per global o-chunk:
        #   PE: delta^T -> PSUM;  ACT: copy -> comb (bf16)
        #   gpsimd DMA accum: comb += base^T (free);  DVE: comb *= mag;  store
        for oc in range(N_OC):
            po = ps.tile([128, 1024], F32, tag="ps", name=f"po_{oc}")
            lhsT = b2f_sb[:, 128 * oc : 128 * (oc + 1)]
            nc.tensor.matmul(
                po[:, 0:512], lhsT, xaT_sb[:, 0:512], start=True, stop=True
            )
            nc.tensor.matmul(
                po[:, 512:1024], lhsT, xaT_sb[:, 512:1024], start=True, stop=True
            )
            comb = combpool.tile([128, 1024], BF16, tag="comb", name=f"comb_{oc}")
            cb, sub = oc // 4, oc % 4
            if oc % 8 < 3:
                # DVE path: fused add straight from PSUM (single-bank reads)
                for h in range(2):
                    nc.vector.tensor_tensor(
                        out=comb[:, 512 * h : 512 * (h + 1)],
                        in0=po[:, 512 * h : 512 * (h + 1)],
                        in1=bt_sb[cb][:, sub, 512 * h : 512 * (h + 1)],
                        op=mybir.AluOpType.add,
                    )
            else:
                # ACT path: copy out of PSUM, then base add on DVE (or DMA)
                nc.scalar.copy(comb[:, 0:512], po[:, 0:512])
                nc.scalar.copy(comb[:, 512:1024], po[:, 512:1024])
                if ACCUM_BASE:
                    nc.gpsimd.dma_start(
                        comb[:], bt_d[oc], accum_op=mybir.AluOpType.add
                    )
                else:
                    nc.vector.tensor_tensor(
                        out=comb[:],
                        in0=comb[:],
                        in1=bt_sb[cb][:, sub, :],
                        op=mybir.AluOpType.add,
                    )
            nc.vector.tensor_scalar_mul(comb[:], comb[:], magb_sb[:, oc : oc + 1])
            nc.sync.dma_start(out_d[oc], comb[:])


def _build():
    nc = bacc.Bacc(
        "TRN2", target_bir_lowering=False, debug=False, num_devices=N_CORES
    )
    aps = {
        "xt": nc.dram_tensor("xt", [N_XC, 128, 4096], BF16, kind="ExternalInput").ap(),
        "bt": nc.dram_tensor("bt", [N_OC, 128, T_LOC], BF16, kind="ExternalInput").ap(),
        "wt": nc.dram_tensor("wt", [128, N_OCL * D], FP8, kind="ExternalInput").ap(),
        "a2": nc.dram_tensor("a2", [R, D], BF16, kind="ExternalInput").ap(),
        "at2": nc.dram_tensor("at2", [128, N_DC * R], BF16, kind="ExternalInput").ap(),
        "b2f": nc.dram_tensor("b2f", [R, O], BF16, kind="ExternalInput").ap(),
        "b2s": nc.dram_tensor("b2s", [R, O_SH], BF16, kind="ExternalInput").ap(),
        "mags": nc.dram_tensor("mags", [128, 4], F32, kind="ExternalInput").ap(),
        "outT": nc.dram_tensor(
            "outT", [N_OC, 128, T_LOC], BF16, kind="ExternalOutput"
        ).ap(),
    }
    with tile.TileContext(nc) as tc:
        _emit(nc, tc, aps)
    nc.compile()
    return nc


def run(inputs: dict, trace: bool = False):
    """Run the SPMD kernel on full inputs; returns (full_output, BassKernelResults)."""
    if "nc" not in _CACHE:
        _CACHE["nc"] = _build()
    nc = _CACHE["nc"]

    x = np.asarray(inputs["x"], dtype=np.float32).reshape(T, D).astype(NP_BF16)
    base = np.asarray(inputs["base_output"], dtype=np.float32).reshape(T, O).astype(
        NP_BF16
    )
    w = np.asarray(inputs["base_weight"], dtype=np.float32)
    a = np.asarray(inputs["lora_A"], dtype=np.float32)
    b = np.asarray(inputs["lora_B"], dtype=np.float32)
    mag = np.asarray(inputs["magnitude"], dtype=np.float32)

    a2 = np.ascontiguousarray((W_SC * SCALING * a).astype(NP_BF16))  # [64, D]
    at2 = (SCALING * a).astype(NP_BF16).T  # [D, 64]
    at2 = np.ascontiguousarray(
        at2.reshape(N_DC, 128, R).transpose(1, 0, 2).reshape(128, N_DC * R)
    )
    b2f = np.ascontiguousarray(b.astype(NP_BF16).T)  # [64, O]

    in_maps = []
    for c in range(N_CORES):
        xs = x[c * T_LOC : (c + 1) * T_LOC]  # [1024, 4096] bf16
        bs = base[c * T_LOC : (c + 1) * T_LOC]
        ws = (W_SC * w[c * O_SH : (c + 1) * O_SH]).astype(NP_FP8)  # [512, 4096]
        in_maps.append(
            {
                "xt": np.ascontiguousarray(
                    xs.T.reshape(N_XC, 4, 128, T_LOC)
                    .transpose(0, 2, 1, 3)
                    .reshape(N_XC, 128, 4096)
                ),
                "bt": np.ascontiguousarray(bs.T.reshape(N_OC, 128, T_LOC)),
                "wt": np.ascontiguousarray(
                    ws.reshape(N_OCL, 128, D).transpose(1, 0, 2).reshape(128, N_OCL * D)
                ),
                "a2": a2,
                "at2": at2,
                "b2f": b2f,
                "b2s": np.ascontiguousarray(b2f[:, c * O_SH : (c + 1) * O_SH]),
                "mags": np.ascontiguousarray(
                    (W_SC * mag[c * O_SH : (c + 1) * O_SH]).reshape(N_OCL, 128).T
                ),
            }
        )

    res = run_bass_kernel_spmd(
        nc, in_maps, core_ids=list(range(N_CORES)), trace=trace
    )
    out = np.empty((T, O), dtype=np.float32)
    for c in range(N_CORES):
        out_t = res.results[c]["outT"].reshape(O, T_LOC).astype(np.float32)
        out[c * T_LOC : (c + 1) * T_LOC] = out_t.T
    return out, res


def kernel(**inputs) -> np.ndarray:
    x = inputs["x"]
    out, _ = run(inputs)
    return out.reshape(x.shape[0], x.shape[1], O).astype(np.float32)


# revision 21
# speedup vs baseline: 1.0554x; 1.0554x over previous
"""DoRA linear kernel for 8 Trainium2 NeuronCores.

out = (base_output + 2.0 * x @ lora_A^T @ lora_B^T) * magnitude / (||base_weight + 2.0 * lora_B @ lora_A||_row + eps)

Sharding (row-parallel hint):
  - tokens (B*S = 8192) data-parallel: 1024 per core (x, base_output, out)
  - base_weight / magnitude row-parallel: 512 out_features per core; the
    per-row norm is fully local, mag_scale is allgathered (16KB collective)
  - lora_A / lora_B replicated

Key design points (all layout transforms done on host):
  - x shipped TRANSPOSED (d-major): stage 1 (xa = 2A @ x^T) needs no PE
    transposes.
  - base_output / out transposed (out_features on partitions): the mag
    rescale is a per-partition DVE tensor_scalar, and the base add costs
    ZERO engine cycles -- a gpsimd software-DGE DMA with accum_op=add
    accumulates base^T straight into the delta tile in SBUF.
  - stage-0 square+rowsum runs on DVE via tensor_tensor_reduce (accum_out),
    keeping ACT free for the epilogue PSUM->SBUF copies.
  - base/out bf16, W fp8-e4m3 scaled by 64 (range fix): 49.5 -> 27.6MB HBM.
  - All tiny-descriptor DMAs eliminated (host pre-tiles magsh; the mag
    collective in/out goes through DVE 32x32 block transposes so every DMA
    runs >= 512B-contiguous descriptors).
  - The collective is triggered as soon as stage 0 drains (~25us) so the
    mag-gated tail (DVE scale + stores) rarely waits.

Engine FIFOs (order == emission order per engine):
  sync : magsh b2s a2 at2 x*8 | maglin | stores*32
  ACT  : W*2 b2f | sqrt | xa copies | comb copies*32
  DVE  : stage0 ttr*16, ss reduce, tail | magb transposes | scale*32
  gpsimd: ident8, cc_in, AllGather | base accum-DMA*32
  PE   : stage0 mm*32, stage1 mm*64, stage2 mm*64
"""

import sys

sys.path.insert(0, "/opt/trn_rl_repo")

import ml_dtypes
import numpy as np

import concourse.bass as bass  # noqa: F401
import concourse.mybir as mybir
import concourse.tile as tile
from concourse import bacc
from concourse.bass_utils import run_bass_kernel_spmd
from concourse.masks import make_identity

N_CORES = 8
T, D, O, R = 8192, 4096, 4096, 64
T_LOC = T // N_CORES  # 1024 tokens per core
O_SH = O // N_CORES  # 512 weight rows per core
SCALING = 2.0
EPS = 1e-8
W_SC = 64.0  # fp8 pre-scale for W (and matching 64x on stage-0 A, mag)
F32 = mybir.dt.float32
BF16 = mybir.dt.bfloat16
FP8 = mybir.dt.float8e4
NP_BF16 = ml_dtypes.bfloat16
NP_FP8 = ml_dtypes.float8_e4m3fn

ACCUM_BASE = False  # add base^T via gpsimd accum-DMA (True) or DVE add (False)
N_OC = O // 128  # 32 global o-chunks (epilogue)
N_OCL = O_SH // 128  # 4 local o-chunks (stage 0)
N_DC = D // 128  # 32 d-chunks (stage 1)
N_XC = 8  # x dma chunks (512 d-rows each)

_CACHE: dict = {}


def _emit(nc, tc, aps):
    xt_d = aps["xt"]  # [8, 128, 4096] bf16  x^T chunks
    bt_d = aps["bt"]  # [32, 128, 1024] bf16 base^T per-oc tiles
    wt_d = aps["wt"]  # [128, 16384] fp8     64*W rows as [128, 4 ocl, 4096]
    a2_d = aps["a2"]  # [64, 4096] bf16      128*A (stage-0 rhs)
    at2_d = aps["at2"]  # [128, 2048] bf16   (2A)^T chunks (stage-1 lhsT)
    b2f_d = aps["b2f"]  # [64, 4096] bf16    B^T full
    b2s_d = aps["b2s"]  # [64, 512] bf16     B^T local o-shard
    mags_d = aps["mags"]  # [128, 4] f32     64*magnitude shard (host-tiled)
    out_d = aps["outT"]  # [32, 128, 1024] bf16 out^T tiles

    import contextlib

    ctx = contextlib.ExitStack()
    with ctx:
        const = ctx.enter_context(tc.tile_pool(name="const", bufs=1))
        combpool = ctx.enter_context(tc.tile_pool(name="combpool", bufs=14))
        ps = ctx.enter_context(tc.tile_pool(name="ps", bufs=3, space="PSUM"))
        pxa = ctx.enter_context(tc.tile_pool(name="pxa", bufs=1, space="PSUM"))
        dram = ctx.enter_context(tc.tile_pool(name="dram", bufs=1, space="DRAM"))

        # ---- phase 0: input DMA triggers
        # sync ring: stage0/1 lora consts, then x^T chunks (8MB)
        b2s_sb = const.tile([64, O_SH], BF16)
        nc.sync.dma_start(b2s_sb[:], b2s_d[:])
        a2_sb = const.tile([64, D], BF16)
        nc.sync.dma_start(a2_sb[:], a2_d[:])
        at2_sb = const.tile([128, N_DC * R], BF16)
        nc.sync.dma_start(at2_sb[:], at2_d[:])
        magsh_sb = const.tile([128, 4], F32)
        nc.sync.dma_start(magsh_sb[:], mags_d[:])
        xt_sb = []
        for g in range(N_XC):
            t = const.tile([128, 4096], BF16, name=f"xt_{g}")
            nc.sync.dma_start(t[:], xt_d[g])
            xt_sb.append(t)

        # scalar ring: W first (gates the collective), then b2f
        w_sb = const.tile([128, N_OCL * D], FP8)
        nc.scalar.dma_start(w_sb[:, 0 : 2 * D], wt_d[:, 0 : 2 * D])
        nc.scalar.dma_start(w_sb[:, 2 * D : 4 * D], wt_d[:, 2 * D : 4 * D])
        b2f_sb = const.tile([64, O], BF16)
        nc.scalar.dma_start(b2f_sb[:], b2f_d[:])
        # base^T tiles: trigger only the first two now -- the other six are
        # emitted after stage 0 so DGE backpressure cannot block ACT's squares
        bt_r = bt_d.rearrange("(c o) p t -> c p o t", o=4)
        bt_sb = []
        for cb in range(N_XC):
            bt_sb.append(const.tile([128, 4, T_LOC], BF16, name=f"bt_{cb}"))
        for cb in range(2):
            nc.scalar.dma_start(bt_sb[cb][:], bt_r[cb])

        ident8 = const.tile([128, 128], FP8)
        make_identity(nc, ident8[:])

        # ---- stage 0: ss = ||64*(W + 2BA)||^2 per local row -> mag_scale
        # dc-pair psum tiles [128,1024]; ACT squares straight from PSUM with
        # the accumulator producing the row-sums (single-bank reads only)
        ss_sb = const.tile([128, 32], F32)
        sqpool = ctx.enter_context(tc.tile_pool(name="sqpool", bufs=3))
        for ocl in range(N_OCL):
            for t4 in range(4):
                pu = ps.tile([128, 1024], F32, tag="ps", name=f"pu_{ocl}_{t4}")
                for h in range(2):
                    dc = 2 * t4 + h
                    nc.tensor.matmul(
                        pu[:, 512 * h : 512 * (h + 1)],
                        b2s_sb[:, 128 * ocl : 128 * (ocl + 1)],
                        a2_sb[:, 512 * dc : 512 * (dc + 1)],
                        start=True,
                        stop=False,
                        skip_group_check=True,
                    )
                for h in range(2):
                    dc = 2 * t4 + h
                    nc.tensor.matmul(
                        pu[:, 512 * h : 512 * (h + 1)],
                        ident8[:],
                        w_sb[:, D * ocl + 512 * dc : D * ocl + 512 * (dc + 1)],
                        start=False,
                        stop=True,
                        skip_group_check=True,
                    )
                for h in range(2):
                    sq = sqpool.tile(
                        [128, 512], BF16, tag="sq", name=f"sq_{ocl}_{t4}_{h}"
                    )
                    k = 8 * ocl + 2 * t4 + h
                    if ocl == 3 or (ocl == 2 and h == 1):
                        # DVE path: bounce, square, rowsum (3 cheap DVE ops)
                        nc.vector.tensor_scalar_mul(
                            sq[:], pu[:, 512 * h : 512 * (h + 1)], 1.0
                        )
                        sq2 = sqpool.tile(
                            [128, 512], BF16, tag="sq2", name=f"sq2_{ocl}_{t4}_{h}"
                        )
                        nc.vector.tensor_tensor(
                            out=sq2[:], in0=sq[:], in1=sq[:],
                            op=mybir.AluOpType.mult,
                        )
                        nc.vector.tensor_reduce(
                            ss_sb[:, k : k + 1],
                            sq2[:],
                            axis=mybir.AxisListType.X,
                            op=mybir.AluOpType.add,
                        )
                    else:
                        nc.scalar.activation(
                            sq[:],
                            pu[:, 512 * h : 512 * (h + 1)],
                            mybir.ActivationFunctionType.Square,
                            accum_out=ss_sb[:, k : k + 1],
                        )

        # remaining base^T loads (ACT queue is past the squares now)
        for cb in range(2, N_XC):
            nc.scalar.dma_start(bt_sb[cb][:], bt_r[cb])

        # tail: magsc = (64*mag) / (sqrt(ss) + 64*eps), then allgather
        ssr_sb = const.tile([128, N_OCL], F32)
        for ocl in range(N_OCL):
            nc.vector.tensor_reduce(
                ssr_sb[:, ocl : ocl + 1],
                ss_sb[:, 8 * ocl : 8 * (ocl + 1)],
                axis=mybir.AxisListType.X,
                op=mybir.AluOpType.add,
            )
        nrm_sb = const.tile([128, N_OCL], F32)
        nc.scalar.sqrt(nrm_sb[:], ssr_sb[:])
        nc.vector.tensor_scalar_add(nrm_sb[:], nrm_sb[:], W_SC * EPS)
        rinv_sb = const.tile([128, N_OCL], F32)
        nc.vector.reciprocal(rinv_sb[:], nrm_sb[:])
        magsc_sb = const.tile([128, N_OCL], F32)
        nc.vector.tensor_tensor(
            out=magsc_sb[:],
            in0=rinv_sb[:],
            in1=magsh_sb[:],
            op=mybir.AluOpType.mult,
        )
        cc_in = dram.tile([O_SH], F32)
        cc_out = dram.tile([O], F32, addr_space="Shared")
        nc.gpsimd.dma_start(cc_in.rearrange("(oc p) -> p oc", p=128), magsc_sb[:])
        nc.gpsimd.collective_compute(
            "AllGather",
            mybir.AluOpType.bypass,
            replica_groups=[list(range(N_CORES))],
            ins=[cc_in[:]],
            outs=[cc_out[:]],
        )
        # [4096] -> [32,128] contiguous load, then block-transpose to [128,32]
        maglin_sb = const.tile([32, 128], F32)
        nc.sync.dma_start(maglin_sb[:], cc_out.rearrange("(q f) -> q f", f=128))
        magb_sb = const.tile([128, N_OC], F32)
        for b in range(4):
            nc.vector.transpose(
                magb_sb[32 * b : 32 * (b + 1), 0:32],
                maglin_sb[0:32, 32 * b : 32 * (b + 1)],
            )

        # ---- stage 1: xa^T[64, 1024] = (2A) @ x^T, accumulated over d
        pxa_t = pxa.tile([64, 1024], F32, name="pxa01")
        pxa0 = pxa_t[:, 0:512]
        pxa1 = pxa_t[:, 512:1024]
        for g in range(N_XC):
            for j in range(4):
                dc = 4 * g + j
                lhsT = at2_sb[:, R * dc : R * (dc + 1)]
                nc.tensor.matmul(
                    pxa0,
                    lhsT,
                    xt_sb[g][:, 1024 * j : 1024 * j + 512],
                    start=(dc == 0),
                    stop=(dc == N_DC - 1),
                )
                nc.tensor.matmul(
                    pxa1,
                    lhsT,
                    xt_sb[g][:, 1024 * j + 512 : 1024 * (j + 1)],
                    start=(dc == 0),
                    stop=(dc == N_DC - 1),
                )
        xaT_sb = const.tile([64, 1024], BF16)
        nc.scalar.copy(xaT_sb[:, 0:512], pxa0)
        nc.scalar.copy(xaT_sb[:, 512:1024], pxa1)

        # ---- stage 2 epilogue, per global o-chunk:
        #   PE: delta^T -> PSUM;  ACT: copy -> comb (bf16)
        #   gpsimd DMA accum: comb += base^T (free);  DVE: comb *= mag;  store
        for oc in range(N_OC):
            po = ps.tile([128, 1024], F32, tag="ps", name=f"po_{oc}")
            lhsT = b2f_sb[:, 128 * oc : 128 * (oc + 1)]
            nc.tensor.matmul(
                po[:, 0:512], lhsT, xaT_sb[:, 0:512], start=True, stop=True
            )
            nc.tensor.matmul(
                po[:, 512:1024], lhsT, xaT_sb[:, 512:1024], start=True, stop=True
            )
            comb = combpool.tile([128, 1024], BF16, tag="comb", name=f"comb_{oc}")
            cb, sub = oc // 4, oc % 4
            if oc % 8 < 3:
                # DVE path: fused add straight from PSUM (single-bank reads)
                for h in range(2):
                    nc.vector.tensor_tensor(
                        out=comb[:, 512 * h : 512 * (h + 1)],
                        in0=po[:, 512 * h : 512 * (h + 1)],
                        in1=bt_sb[cb][:, sub, 512 * h : 512 * (h + 1)],
                        op=mybir.AluOpType.add,
                    )
            else:
                # ACT path: copy out of PSUM, then base add on DVE (or DMA)
                nc.scalar.copy(comb[:, 0:512], po[:, 0:512])
                nc.scalar.copy(comb[:, 512:1024], po[:, 512:1024])
                if ACCUM_BASE:
                    nc.gpsimd.dma_start(
                        comb[:], bt_d[oc], accum_op=mybir.AluOpType.add
                    )
                else:
                    nc.vector.tensor_tensor(
                        out=comb[:],
                        in0=comb[:],
                        in1=bt_sb[cb][:, sub, :],
                        op=mybir.AluOpType.add,
                    )
            nc.vector.tensor_scalar_mul(comb[:], comb[:], magb_sb[:, oc : oc + 1])
            nc.sync.dma_start(out_d[oc], comb[:])


def _build():
    nc = bacc.Bacc(
        "TRN2", target_bir_lowering=False, debug=False, num_devices=N_CORES
    )
    aps = {
        "xt": nc.dram_tensor("xt", [N_XC, 128, 4096], BF16, kind="ExternalInput").ap(),
        "bt": nc.dram_tensor("bt", [N_OC, 128, T_LOC], BF16, kind="ExternalInput").ap(),
        "wt": nc.dram_tensor("wt", [128, N_OCL * D], FP8, kind="ExternalInput").ap(),
        "a2": nc.dram_tensor("a2", [R, D], BF16, kind="ExternalInput").ap(),
        "at2": nc.dram_tensor("at2", [128, N_DC * R], BF16, kind="ExternalInput").ap(),
        "b2f": nc.dram_tensor("b2f", [R, O], BF16, kind="ExternalInput").ap(),
        "b2s": nc.dram_tensor("b2s", [R, O_SH], BF16, kind="ExternalInput").ap(),
        "mags": nc.dram_tensor("mags", [128, 4], F32, kind="ExternalInput").ap(),
        "outT": nc.dram_tensor(
            "outT", [N_OC, 128, T_LOC], BF16, kind="ExternalOutput"
        ).ap(),
    }
    with tile.TileContext(nc) as tc:
        _emit(nc, tc, aps)
    nc.compile()
    return nc


def run(inputs: dict, trace: bool = False):
    """Run the SPMD kernel on full inputs; returns (full_output, BassKernelResults)."""
    if "nc" not in _CACHE:
        _CACHE["nc"] = _build()
    nc = _CACHE["nc"]

    x = np.asarray(inputs["x"], dtype=np.float32).reshape(T, D).astype(NP_BF16)
    base = np.asarray(inputs["base_output"], dtype=np.float32).reshape(T, O).astype(
        NP_BF16
    )
    w = np.asarray(inputs["base_weight"], dtype=np.float32)
    a = np.asarray(inputs["lora_A"], dtype=np.float32)
    b = np.asarray(inputs["lora_B"], dtype=np.float32)
    mag = np.asarray(inputs["magnitude"], dtype=np.float32)

    a2 = np.ascontiguousarray((W_SC * SCALING * a).astype(NP_BF16))  # [64, D]
    at2 = (SCALING * a).astype(NP_BF16).T  # [D, 64]
    at2 = np.ascontiguousarray(
        at2.reshape(N_DC, 128, R).transpose(1, 0, 2).reshape(128, N_DC * R)
    )
    b2f = np.ascontiguousarray(b.astype(NP_BF16).T)  # [64, O]

    in_maps = []
    for c in range(N_CORES):
        xs = x[c * T_LOC : (c + 1) * T_LOC]  # [1024, 4096] bf16
        bs = base[c * T_LOC : (c + 1) * T_LOC]
        ws = (W_SC * w[c * O_SH : (c + 1) * O_SH]).astype(NP_FP8)  # [512, 4096]
        in_maps.append(
            {
                "xt": np.ascontiguousarray(
                    xs.T.reshape(N_XC, 4, 128, T_LOC)
                    .transpose(0, 2, 1, 3)
                    .reshape(N_XC, 128, 4096)
                ),
                "bt": np.ascontiguousarray(bs.T.reshape(N_OC, 128, T_LOC)),
                "wt": np.ascontiguousarray(
                    ws.reshape(N_OCL, 128, D).transpose(1, 0, 2).reshape(128, N_OCL * D)
                ),
                "a2": a2,
                "at2": at2,
                "b2f": b2f,
                "b2s": np.ascontiguousarray(b2f[:, c * O_SH : (c + 1) * O_SH]),
                "mags": np.ascontiguousarray(
                    (W_SC * mag[c * O_SH : (c + 1) * O_SH]).reshape(N_OCL, 128).T
                ),
            }
        )

    res = run_bass_kernel_spmd(
        nc, in_maps, core_ids=list(range(N_CORES)), trace=trace
    )
    out = np.empty((T, O), dtype=np.float32)
    for c in range(N_CORES):
        out_t = res.results[c]["outT"].reshape(O, T_LOC).astype(np.float32)
        out[c * T_LOC : (c + 1) * T_LOC] = out_t.T
    return out, res


def kernel(**inputs) -> np.ndarray:
    x = inputs["x"]
    out, _ = run(inputs)
    return out.reshape(x.shape[0], x.shape[1], O).astype(np.float32)
